# revision 29
# baseline (speedup 1.0000x reference)
"""Trainium2 Bass kernel for nn_Attention_61177514164290.

Gated multi-head attention with RoPE:
  qkv = x @ w_qkv ; rope(q), rope(k) ; attn = softmax(q k^T / 8)
  out = (attn @ v) * sigmoid(x @ w_gates + b_gates) ; out @ w_out + b_out

Sharding: row-parallel over (batch, query-rows). 8 cores, core c owns batch
c//4 and query rows [(c%4)*1024, +1024) for ALL 8 heads. K/V projections are
recomputed per core for its batch (cheaper than any inter-core collective on
this chip), so there are ZERO collectives; the host concatenates the 8 row
slices.

Per-core dataflow (all matmuls bf16 with f32 PSUM accumulation):
  - x^T tiles [128d, N] -> K^T/Q^T projections with host-permuted weights
    (even dh dims then odd) so RoPE pairs become 32-row partition blocks.
  - RoPE = norm .* C + swap .* S on DVE, where `swap` is a partition-block
    swapped copy made by SBUF->SBUF DMA and S has the -sin/+sin signs baked.
  - S^T tile [128 keys, 512 q] = (K~ pair slice [64,128]).T @ Q~ [64, 512];
    exp on the Scalar engine reads 3 PSUM banks per instruction (scale=1/8,
    no max-subtraction needed: |s|/8 < ~6), writes P~ bf16 to SBUF.
  - O^T [65, 512] accumulates ([V_h | 1]).T @ P~ over 32 keytiles in a single
    full-bank PSUM region (start=True resets accumulation state bank-wide,
    so per-qtile regions must not interleave); row 64 = softmax denominators.
  - alpha row = gate^T[h] / denominators (DVE), DMA-broadcast across 64
    partitions, one DVE multiply scales O^T into the out-projection layout.
  - out-projection packs head-pairs to K=128, + b_out, DMA out.
"""

import sys

for _p in ("/opt/trn_rl_repo",):
    if _p not in sys.path:
        sys.path.insert(0, _p)

import numpy as np
import ml_dtypes

import concourse.bass as bass
from concourse import mybir
from concourse.bass_utils import run_bass_kernel_spmd

BF16 = mybir.dt.bfloat16
F32 = mybir.dt.float32
EXP = mybir.ActivationFunctionType.Exp

N_CORES = 8
HEADS = 8
DIM_HEAD = 64
MAX_FREQ = 10.0


def _split(n, size):
    out = []
    i = 0
    while i < n:
        out.append((i, min(size, n - i)))
        i += size
    return out


def build_nc(D=512, N=4096, R=1024, H=8, DOUT=512, phase=6):
    DT = D // 128          # d-tiles
    KT = N // 128          # key tiles
    PAIRS = H // 2
    dh = DIM_HEAD

    nc = bass.Bass()

    # ---------------- DRAM parameters ----------------
    xt_d = nc.dram_tensor("xt", [D, N], BF16, kind="ExternalInput")
    xq_d = nc.dram_tensor("xq", [D, R], BF16, kind="ExternalInput")
    wk_d = nc.dram_tensor("wk", [PAIRS, DT, 128, 128], BF16, kind="ExternalInput")
    wq_d = nc.dram_tensor("wq", [PAIRS, DT, 128, 128], BF16, kind="ExternalInput")
    wv_d = nc.dram_tensor("wv", [DT, 128, H * dh], BF16, kind="ExternalInput")
    wg_d = nc.dram_tensor("wg", [DT, 128, H], BF16, kind="ExternalInput")
    wo_d = nc.dram_tensor("wo", [PAIRS, 128, DOUT], BF16, kind="ExternalInput")
    bgn_d = nc.dram_tensor("bgn", [128, 1], F32, kind="ExternalInput")
    bo_d = nc.dram_tensor("bo", [128, DOUT], F32, kind="ExternalInput")
    cos_d = nc.dram_tensor("cost", [128, N], BF16, kind="ExternalInput")
    sin_d = nc.dram_tensor("sint", [128, N], BF16, kind="ExternalInput")
    cosq_d = nc.dram_tensor("cosq", [128, R], BF16, kind="ExternalInput")
    sinq_d = nc.dram_tensor("sinq", [128, R], BF16, kind="ExternalInput")
    out_d = nc.dram_tensor("out", [R, DOUT], F32, kind="ExternalOutput")
    alb_d = nc.dram_tensor("alb", [1, 512], F32)
    gd_d = nc.dram_tensor("gd", [H, R], F32)

    # ---------------- SBUF ----------------
    xt_s = [nc.alloc_sbuf_tensor(f"xt{i}", [128, N], BF16) for i in range(DT)]
    xq_s = [nc.alloc_sbuf_tensor(f"xq{i}", [128, R], BF16) for i in range(DT)]
    wk_s = [[nc.alloc_sbuf_tensor(f"wk{j}_{i}", [128, 128], BF16) for i in range(DT)]
            for j in range(PAIRS)]
    wq_s = [[nc.alloc_sbuf_tensor(f"wq{j}_{i}", [128, 128], BF16) for i in range(DT)]
            for j in range(PAIRS)]
    wv_s = [nc.alloc_sbuf_tensor(f"wv{i}", [128, H * dh], BF16) for i in range(DT)]
    wg_s = [nc.alloc_sbuf_tensor(f"wg{i}", [128, H], BF16) for i in range(DT)]
    wo_s = [nc.alloc_sbuf_tensor(f"wo{j}", [128, DOUT], BF16) for j in range(PAIRS)]
    bgn_s = nc.alloc_sbuf_tensor("bgns", [128, 1], F32)
    bo_s = nc.alloc_sbuf_tensor("bos", [128, DOUT], F32)
    cos_s = nc.alloc_sbuf_tensor("coss", [128, N], BF16)
    sin_s = nc.alloc_sbuf_tensor("sins", [128, N], BF16)
    cosq_s = nc.alloc_sbuf_tensor("cosqs", [128, R], BF16)
    sinq_s = nc.alloc_sbuf_tensor("sinqs", [128, R], BF16)

    kn_s = [nc.alloc_sbuf_tensor(f"kn{j}", [128, N], BF16) for j in range(PAIRS)]
    ks_s = [nc.alloc_sbuf_tensor(f"ks{j}", [128, N], BF16) for j in range(min(2, PAIRS))]
    qn_s = [nc.alloc_sbuf_tensor(f"qn{j}", [128, R], BF16) for j in range(PAIRS)]
    qs_s = [nc.alloc_sbuf_tensor(f"qs{j}", [128, R], BF16) for j in range(min(2, PAIRS))]
    v_s = [nc.alloc_sbuf_tensor(f"v{t}", [128, H * 65], BF16) for t in range(KT)]

    QCHUNKS = _split(R, 512)          # (start, len) query chunks
    QCLEN = max(l for _, l in QCHUNKS)
    assert all(l == QCLEN for _, l in QCHUNKS), "uniform q chunks required"
    GROUPS = _split(KT, 3)            # (kt0, ngroup) keytile groups
    NG = len(GROUPS)

    p_sbuf = [nc.alloc_sbuf_tensor(f"p{i}", [128, QCLEN * 3], BF16) for i in range(2)]
    oT_s = [nc.alloc_sbuf_tensor(f"oT{j}", [128, QCLEN], BF16) for j in range(PAIRS)]
    gT_s = nc.alloc_sbuf_tensor("gT", [H, R], F32)
    al_s = nc.alloc_sbuf_tensor("al", [1, QCLEN], F32)
    al2_s = nc.alloc_sbuf_tensor("al2", [1, QCLEN], F32)
    alr_s = nc.alloc_sbuf_tensor("alr", [64, QCLEN], F32)
    outb_s = [nc.alloc_sbuf_tensor(f"outb{i}", [128, DOUT], F32) for i in range(2)]

    # ---------------- PSUM (8 banks) ----------------
    s_ps = [nc.alloc_psum_tensor(f"sps{i}", [128, QCLEN * 3], F32) for i in range(2)]
    o_ps = nc.alloc_psum_tensor("ops", [128, 512], F32)
    t_ps = nc.alloc_psum_tensor("tps", [128, 512], F32)

    # proj-phase views of the S psum banks
    proj_ps = [s_ps[0].ap()[:, 0:512], s_ps[0].ap()[:, 512:1024]]
    g_ps = s_ps[1].ap()[0:H, 0:512]

    # ---------------- shared schedules ----------------
    proj_chunks = []                  # (kind, j/tt, col_start, col_len)
    for j in range(PAIRS):
        for c0, cl in _split(N, 512):
            proj_chunks.append(("K", j, c0, cl))
    for j in range(PAIRS):
        for c0, cl in _split(R, 512):
            proj_chunks.append(("Q", j, c0, cl))
    for tt in range(KT):
        proj_chunks.append(("V", tt, 0, H * dh))
    GCHUNKS = _split(R, 512)
    for gi, (c0, cl) in enumerate(GCHUNKS):
        proj_chunks.append(("G", gi, c0, cl))
    NPROJ = len(proj_chunks)
    NCOPY = NPROJ - len(GCHUNKS)       # dve_copy counts K/Q/V chunks only

    n_in_dma = DT + DT + PAIRS * DT * 2 + DT + DT + PAIRS + 6
    NROPE = PAIRS * 3 * 2
    NIDX = len(QCHUNKS) * H            # (qc, h) pairs

    sems = {}

    def S(name):
        return sems[name]

    import contextlib
    es = contextlib.ExitStack()
    for name in ["in_sem", "swap_sem", "outd_sem", "brd_sem", "pe_proj", "pe_s",
                 "pe_av", "pe_out", "act_s", "act_g", "dve_copy", "dve_rope",
                 "dve_al", "dve_ot", "dve_badd", "alb_sem", "g2_sem", "gd_sem", "dve_gfin"]:
        sems[name] = es.enter_context(nc.semaphore(name))

    with nc.Block() as block:

        # ================= SP: DMAs =================
        @block.sync
        def _(sp):
            def ld(dst, src):
                sp.dma_start(out=dst, in_=src).then_inc(S("in_sem"), 16)

            for i in range(DT):
                ld(xt_s[i].ap(), xt_d.ap()[i * 128:(i + 1) * 128, :])
            for i in range(DT):
                ld(xq_s[i].ap(), xq_d.ap()[i * 128:(i + 1) * 128, :])
            for j in range(PAIRS):
                for i in range(DT):
                    ld(wk_s[j][i].ap(), wk_d.ap()[j, i])
                    ld(wq_s[j][i].ap(), wq_d.ap()[j, i])
            for i in range(DT):
                ld(wv_s[i].ap(), wv_d.ap()[i])
                ld(wg_s[i].ap(), wg_d.ap()[i])
            for j in range(PAIRS):
                ld(wo_s[j].ap(), wo_d.ap()[j])
            ld(bgn_s.ap(), bgn_d.ap())
            ld(bo_s.ap(), bo_d.ap())
            ld(cos_s.ap(), cos_d.ap())
            ld(sin_s.ap(), sin_d.ap())
            ld(cosq_s.ap(), cosq_d.ap())
            ld(sinq_s.ap(), sinq_d.ap())

            if phase < 2:
                return
            # swap copies for rope (partition-block swap via DMA)
            nk = PAIRS * len(_split(N, 512))
            nq = PAIRS * len(_split(R, 512))
            sp.wait_ge(S("dve_copy"), nk)
            nb = min(2, PAIRS)
            for j in range(PAIRS):
                if j >= nb:
                    sp.wait_ge(S("dve_rope"), 3 * (j - nb + 1))
                for a, b in ((0, 32), (32, 0), (64, 96), (96, 64)):
                    sp.dma_start(
                        out=ks_s[j % nb].ap()[b:b + 32, :],
                        in_=kn_s[j].ap()[a:a + 32, :],
                    ).then_inc(S("swap_sem"), 16)
            sp.wait_ge(S("dve_copy"), nk + nq)
            for j in range(PAIRS):
                if j >= nb:
                    sp.wait_ge(S("dve_rope"), 3 * PAIRS + 3 * (j - nb + 1))
                for a, b in ((0, 32), (32, 0), (64, 96), (96, 64)):
                    sp.dma_start(
                        out=qs_s[j % nb].ap()[b:b + 32, :],
                        in_=qn_s[j].ap()[a:a + 32, :],
                    ).then_inc(S("swap_sem"), 16)

            if phase < 5:
                return
            # gates to DRAM, alpha broadcasts via DRAM bounce, output stores
            sp.wait_ge(S("dve_gfin"), 1)
            sp.dma_start(out=gd_d.ap(), in_=gT_s.ap()).then_inc(S("gd_sem"), 16)
            sp.wait_ge(S("gd_sem"), 16)
            alb_ap = alb_d.ap()[:, 0:QCLEN]
            alr_bcast = bass.AP(
                tensor=alb_ap.tensor, offset=alb_ap.offset,
                ap=[[0, 64]] + alb_ap.ap[1:])
            for qci, (q0, qcl) in enumerate(QCHUNKS):
                for h in range(H):
                    idx = qci * H + h
                    sp.wait_ge(S("dve_al"), idx)         # al2 consumed
                    sp.dma_start(out=al2_s.ap()[:, 0:qcl],
                                 in_=gd_d.ap()[h:h + 1, q0:q0 + qcl]).then_inc(
                        S("g2_sem"), 16)
                    sp.wait_ge(S("dve_al"), idx + 1)
                    sp.dma_start(out=alb_ap, in_=al_s.ap()).then_inc(
                        S("alb_sem"), 16)
                    sp.wait_ge(S("alb_sem"), 16 * (idx + 1))
                    if idx >= 1:
                        sp.wait_ge(S("dve_ot"), idx)     # alr consumed
                    sp.dma_start(out=alr_s.ap(), in_=alr_bcast).then_inc(
                        S("brd_sem"), 16)
                if phase < 6:
                    continue
                for qt in range(qcl // 128):
                    gq = q0 // 128 + qt
                    sp.wait_ge(S("dve_badd"), gq + 1)
                    sp.dma_start(
                        out=out_d.ap()[gq * 128:(gq + 1) * 128, :],
                        in_=outb_s[gq % 2].ap(),
                    ).then_inc(S("outd_sem"), 16)

        # ================= PE =================
        @block.tensor
        def _(pe):
            pe.wait_ge(S("in_sem"), 16 * n_in_dma)

            # ---- projections ----
            gidx = 0
            for i, (kind, jt, c0, cl) in enumerate(proj_chunks):
                if kind == "G":
                    if gidx >= 1:
                        pe.wait_ge(S("act_g"), gidx)     # g_ps reused
                elif i >= 2:
                    pe.wait_ge(S("dve_copy"), min(i - 1, NCOPY))
                buf = proj_ps[i % 2]
                for dt in range(DT):
                    if kind == "K":
                        o, l, r = buf[:, 0:cl], wk_s[jt][dt].ap(), xt_s[dt].ap()[:, c0:c0 + cl]
                    elif kind == "Q":
                        o, l, r = buf[:, 0:cl], wq_s[jt][dt].ap(), xq_s[dt].ap()[:, c0:c0 + cl]
                    elif kind == "V":
                        o, l, r = buf[:, 0:cl], xt_s[dt].ap()[:, jt * 128:(jt + 1) * 128], wv_s[dt].ap()
                    else:  # G: gates^T [H, cl]
                        o, l, r = g_ps[:, 0:cl], wg_s[dt].ap(), xq_s[dt].ap()[:, c0:c0 + cl]
                    ins = pe.matmul(o, l, r, start=(dt == 0), stop=(dt == DT - 1))
                ins.then_inc(S("pe_proj"), 1)
                if kind == "G":
                    gidx += 1

            # ---- attention ----
            if phase < 3:
                return
            pe.wait_ge(S("dve_rope"), NROPE)
            pe.wait_ge(S("dve_copy"), NCOPY)
            if phase >= 2:
                pe.wait_ge(S("act_g"), len(GCHUNKS))   # g_ps bank reused by S groups

            Gg = 0
            for qci, (q0, qcl) in enumerate(QCHUNKS):
                for h in range(H):
                    idx = qci * H + h
                    j, par = h // 2, h % 2
                    base = 64 * par

                    def s_group(g):
                        kt0, ng = GROUPS[g]
                        gg = Gg + g
                        for l in range(ng):
                            kt = kt0 + l
                            ins = pe.matmul(
                                s_ps[gg % 2].ap()[:, qcl * l: qcl * (l + 1)],
                                kn_s[j].ap()[base:base + 64, kt * 128:(kt + 1) * 128],
                                qn_s[j].ap()[base:base + 64, q0:q0 + qcl],
                            )
                        ins.then_inc(S("pe_s"), 1)

                    s_group(0)
                    if NG > 1:
                        s_group(1)
                    for g in range(NG):
                        kt0, ng = GROUPS[g]
                        gg = Gg + g
                        pe.wait_ge(S("act_s"), gg + 1)
                        if g == 0 and phase >= 5 and idx >= 1:
                            pe.wait_ge(S("dve_ot"), idx)     # o_ps consumed
                        last = None
                        if phase >= 4:
                            for l in range(ng):
                                kt = kt0 + l
                                last = pe.matmul(
                                    o_ps.ap()[0:65, 0:qcl],
                                    v_s[kt].ap()[:, 65 * h: 65 * h + 65],
                                    p_sbuf[gg % 2].ap()[:, qcl * l: qcl * (l + 1)],
                                    start=(kt == 0), stop=(kt == KT - 1),
                                    skip_group_check=True,
                                )
                        if g + 2 < NG:
                            s_group(g + 2)
                        if phase >= 4 and g == NG - 1:
                            last.then_inc(S("pe_av"), 1)

                    Gg += NG

                # out-projection for this q chunk
                if phase < 6:
                    continue
                pe.wait_ge(S("dve_ot"), (qci + 1) * H)
                for qt in range(qcl // 128):
                    gq = q0 // 128 + qt
                    pe.wait_ge(S("dve_badd"), gq)        # t_ps bank free
                    for j2 in range(PAIRS):
                        ins = pe.matmul(
                            t_ps.ap()[:, 0:DOUT],
                            oT_s[j2].ap()[:, 128 * qt:128 * (qt + 1)],
                            wo_s[j2].ap(),
                            start=(j2 == 0), stop=(j2 == PAIRS - 1),
                        )
                    ins.then_inc(S("pe_out"), 1)

        # ================= ACT =================
        @block.scalar
        def _(act):
            if phase < 2:
                return
            # gates: u = exp(-(z + bg)) straight from PSUM, per-partition bias
            ng_chunk_base = NPROJ - len(GCHUNKS)
            for gi, (c0, cl) in enumerate(GCHUNKS):
                act.wait_ge(S("pe_proj"), ng_chunk_base + gi + 1)
                act.activation(
                    gT_s.ap()[:, c0:c0 + cl], g_ps[:, 0:cl], EXP,
                    bias=bgn_s.ap()[0:H, :], scale=-1.0,
                ).then_inc(S("act_g"), 1)

            if phase < 3:
                return
            for gg in range(NIDX * NG):
                g = gg % NG
                qci = gg // (NG * H)
                _, qcl = QCHUNKS[qci]
                _, ngrp = GROUPS[g]
                act.wait_ge(S("pe_s"), gg + 1)
                act.activation(
                    p_sbuf[gg % 2].ap()[:, 0:qcl * ngrp],
                    s_ps[gg % 2].ap()[:, 0:qcl * ngrp],
                    EXP, scale=0.125,
                ).then_inc(S("act_s"), 1)

        # ================= DVE =================
        @block.vector
        def _(dve):
            dve.wait_ge(S("in_sem"), 16 * n_in_dma)

            # proj copies (K/Q/V only; gates go straight PSUM->ACT)
            for i, (kind, jt, c0, cl) in enumerate(proj_chunks):
                if kind == "G":
                    continue
                dve.wait_ge(S("pe_proj"), i + 1)
                buf = proj_ps[i % 2]
                if kind == "K":
                    ins = dve.tensor_copy(kn_s[jt].ap()[:, c0:c0 + cl], buf[:, 0:cl])
                elif kind == "Q":
                    ins = dve.tensor_copy(qn_s[jt].ap()[:, c0:c0 + cl], buf[:, 0:cl])
                else:  # V
                    vt3 = v_s[jt].ap().rearrange("p (h c) -> p h c", c=65)
                    dve.memset(vt3[:, :, 64:65], 1.0)
                    for h in range(H):
                        ins = dve.tensor_copy(
                            vt3[:, h, 0:dh], buf[:, h * dh:(h + 1) * dh])
                ins.then_inc(S("dve_copy"), 1)

            if phase < 2:
                return
            # rope: x~ = norm*C + swap*S   (drain: same-engine RAW)
            nb = min(2, PAIRS)
            for j in range(PAIRS):
                dve.wait_ge(S("swap_sem"), 16 * (j + 1) * 4)
                dve.tensor_mul(kn_s[j].ap(), kn_s[j].ap(), cos_s.ap()).then_inc(S("dve_rope"), 1)
                dve.tensor_mul(ks_s[j % nb].ap(), ks_s[j % nb].ap(), sin_s.ap()).then_inc(S("dve_rope"), 1)
                dve.drain()
                dve.tensor_add(kn_s[j].ap(), kn_s[j].ap(), ks_s[j % nb].ap()).then_inc(S("dve_rope"), 1)
            for j in range(PAIRS):
                dve.wait_ge(S("swap_sem"), 16 * (PAIRS * 4 + (j + 1) * 4))
                dve.tensor_mul(qn_s[j].ap(), qn_s[j].ap(), cosq_s.ap()).then_inc(S("dve_rope"), 1)
                dve.tensor_mul(qs_s[j % nb].ap(), qs_s[j % nb].ap(), sinq_s.ap()).then_inc(S("dve_rope"), 1)
                dve.drain()
                dve.tensor_add(qn_s[j].ap(), qn_s[j].ap(), qs_s[j % nb].ap()).then_inc(S("dve_rope"), 1)

            # gates finish: g = 1 / (1 + u)
            dve.wait_ge(S("act_g"), len(GCHUNKS))
            dve.tensor_scalar_add(gT_s.ap(), gT_s.ap(), 1.0)
            dve.drain()
            dve.reciprocal(gT_s.ap(), gT_s.ap()).then_inc(S("dve_gfin"), 1)

            # attention epilogues
            if phase < 5:
                return
            for qci, (q0, qcl) in enumerate(QCHUNKS):
                for h in range(H):
                    idx = qci * H + h
                    j, par = h // 2, h % 2
                    base = 64 * par
                    dve.wait_ge(S("pe_av"), idx + 1)
                    dve.reciprocal(al_s.ap(), o_ps.ap()[64:65, 0:qcl])
                    dve.drain()
                    dve.wait_ge(S("g2_sem"), 16 * (idx + 1))
                    dve.tensor_mul(al_s.ap(), al_s.ap(),
                                   al2_s.ap()[:, 0:qcl]).then_inc(S("dve_al"), 1)
                    dve.wait_ge(S("brd_sem"), 16 * (idx + 1))
                    dve.tensor_mul(
                        oT_s[j].ap()[base:base + 64, 0:qcl],
                        o_ps.ap()[0:64, 0:qcl],
                        alr_s.ap(),
                    ).then_inc(S("dve_ot"), 1)

                if phase < 6:
                    continue
                for qt in range(qcl // 128):
                    gq = q0 // 128 + qt
                    dve.wait_ge(S("pe_out"), gq + 1)
                    if gq >= 2:
                        dve.wait_ge(S("outd_sem"), 16 * (gq - 1))
                    dve.tensor_add(
                        outb_s[gq % 2].ap(), t_ps.ap()[:, 0:DOUT], bo_s.ap(),
                    ).then_inc(S("dve_badd"), 1)

    es.close()
    return nc


# =====================================================================
# Host side
# =====================================================================

_CACHED = {}


def _rope_tables(n):
    half = DIM_HEAD // 2                      # 32
    inv_freq = 1.0 / (MAX_FREQ ** (np.arange(0, half, 2, dtype=np.float32) / half))
    invf = np.concatenate([inv_freq, inv_freq])            # [32]
    t = np.arange(n, dtype=np.float32)
    ang = invf[:, None] * t[None, :]                       # [32, n]
    return np.cos(ang), np.sin(ang)


def _prep_shared(wq_full, wk_full, wv_full, wg_full, bg_full, wo_full, bo_full,
                 N, H=HEADS):
    dh = DIM_HEAD
    D = wq_full.shape[0]
    DT = D // 128
    PAIRS = H // 2
    perm = np.concatenate([np.arange(0, dh, 2), np.arange(1, dh, 2)])
    bf = ml_dtypes.bfloat16

    def pack_pairs(w):
        arr = np.zeros((PAIRS, DT, 128, 128), np.float32)
        for j in range(PAIRS):
            pair = np.concatenate(
                [w[:, 64 * (2 * j + p):64 * (2 * j + p) + 64][:, perm]
                 for p in (0, 1)], axis=1)               # [D, 128]
            for dt in range(DT):
                arr[j, dt] = pair[128 * dt:128 * (dt + 1), :]
        return arr.astype(bf)

    wk = pack_pairs(wk_full)
    wq = pack_pairs(wq_full)
    wv = np.stack([wv_full[128 * dt:128 * (dt + 1), :] for dt in range(DT)]).astype(bf)
    wg = np.stack([wg_full[128 * dt:128 * (dt + 1), :] for dt in range(DT)]).astype(bf)
    wo = np.stack([wo_full[128 * j:128 * (j + 1), :] for j in range(PAIRS)]).astype(bf)
    bgn = np.zeros((128, 1), np.float32)
    bgn[:H, 0] = -bg_full
    bo = np.tile(bo_full[None, :], (128, 1)).astype(np.float32)

    cos32, sin32 = _rope_tables(N)                         # [32, N]
    cost = np.tile(cos32, (4, 1)).astype(bf)               # [128, N]
    sint = np.tile(np.concatenate([-sin32, sin32], 0), (2, 1)).astype(bf)
    return dict(wk=wk, wq=wq, wv=wv, wg=wg, wo=wo, bgn=bgn, bo=bo,
                cost=cost, sint=sint)


def kernel(x, w_qkv, w_gates, b_gates, w_out, b_out):
    b, n, d = x.shape
    R = (b * n) // N_CORES
    bf = ml_dtypes.bfloat16

    key = ("nc", d, n, R)
    if key not in _CACHED:
        _CACHED[key] = build_nc(D=d, N=n, R=R, H=HEADS, DOUT=d)
    nc = _CACHED[key]

    w_qkv = np.asarray(w_qkv, np.float32)
    inner = HEADS * DIM_HEAD
    shared = _prep_shared(
        w_qkv[:, 0:inner], w_qkv[:, inner:2 * inner], w_qkv[:, 2 * inner:3 * inner],
        np.asarray(w_gates, np.float32), np.asarray(b_gates, np.float32),
        np.asarray(w_out, np.float32), np.asarray(b_out, np.float32), n)

    per_batch = N_CORES // b
    in_maps = []
    for c in range(N_CORES):
        bi = c // per_batch
        q0 = (c % per_batch) * R
        xt = np.ascontiguousarray(np.asarray(x[bi], np.float32).T).astype(bf)
        xq = np.ascontiguousarray(xt[:, q0:q0 + R])
        in_maps.append(dict(
            xt=xt, xq=xq,
            cosq=np.ascontiguousarray(shared["cost"][:, q0:q0 + R]),
            sinq=np.ascontiguousarray(shared["sint"][:, q0:q0 + R]),
            **shared))

    res = run_bass_kernel_spmd(nc, in_maps, core_ids=list(range(N_CORES)))
    out = np.concatenate([res.results[c]["out"] for c in range(N_CORES)], axis=0)
    return out.reshape(b, n, d).astype(np.float32)


# revision 30
# speedup vs baseline: 1.2635x; 1.2635x over previous
"""Trainium2 Bass kernel for nn_Attention_61177514164290.

Gated multi-head attention with RoPE:
  qkv = x @ w_qkv ; rope(q), rope(k) ; attn = softmax(q k^T / 8)
  out = (attn @ v) * sigmoid(x @ w_gates + b_gates) ; out @ w_out + b_out

Sharding: row-parallel over (batch, query-rows). 8 cores, core c owns batch
c//4 and query rows [(c%4)*1024, +1024) for ALL 8 heads. K/V projections are
recomputed per core for its batch (cheaper than any inter-core collective on
this chip), so there are ZERO collectives; the host concatenates the 8 row
slices.

Per-core dataflow (all matmuls bf16 with f32 PSUM accumulation):
  - x^T tiles [128d, N] -> K^T/Q^T projections with host-permuted weights
    (even dh dims then odd) so RoPE pairs become 32-row partition blocks.
  - RoPE = norm .* C + swap .* S on DVE, where `swap` is a partition-block
    swapped copy made by SBUF->SBUF DMA and S has the -sin/+sin signs baked.
  - S^T tile [128 keys, 512 q] = (K~ pair slice [64,128]).T @ Q~ [64, 512];
    exp on the Scalar engine reads 3 PSUM banks per instruction (scale=1/8,
    no max-subtraction needed: |s|/8 < ~6), writes P~ bf16 to SBUF.
  - O^T [65, 512] accumulates ([V_h | 1]).T @ P~ over 32 keytiles in a single
    full-bank PSUM region (start=True resets accumulation state bank-wide,
    so per-qtile regions must not interleave); row 64 = softmax denominators.
  - alpha row = gate^T[h] / denominators (DVE), DMA-broadcast across 64
    partitions, one DVE multiply scales O^T into the out-projection layout.
  - out-projection packs head-pairs to K=128, + b_out, DMA out.
"""

import sys

for _p in ("/opt/trn_rl_repo",):
    if _p not in sys.path:
        sys.path.insert(0, _p)

import numpy as np
import ml_dtypes

import concourse.bass as bass
from concourse import mybir
from concourse.bass_utils import run_bass_kernel_spmd

BF16 = mybir.dt.bfloat16
F32 = mybir.dt.float32
EXP = mybir.ActivationFunctionType.Exp

N_CORES = 8
HEADS = 8
DIM_HEAD = 64
MAX_FREQ = 10.0


def _split(n, size):
    out = []
    i = 0
    while i < n:
        out.append((i, min(size, n - i)))
        i += size
    return out


def build_nc(D=512, N=4096, R=1024, H=8, DOUT=512, phase=6):
    DT = D // 128          # d-tiles
    KT = N // 128          # key tiles
    PAIRS = H // 2
    dh = DIM_HEAD

    nc = bass.Bass()

    # ---------------- DRAM parameters ----------------
    xt_d = nc.dram_tensor("xt", [D, N], BF16, kind="ExternalInput")
    xq_d = nc.dram_tensor("xq", [D, R], BF16, kind="ExternalInput")
    wk_d = nc.dram_tensor("wk", [PAIRS, DT, 128, 128], BF16, kind="ExternalInput")
    wq_d = nc.dram_tensor("wq", [PAIRS, DT, 128, 128], BF16, kind="ExternalInput")
    wv_d = nc.dram_tensor("wv", [DT, 128, H * dh], BF16, kind="ExternalInput")
    wg_d = nc.dram_tensor("wg", [DT, 128, H], BF16, kind="ExternalInput")
    wo_d = nc.dram_tensor("wo", [PAIRS, 128, DOUT], BF16, kind="ExternalInput")
    bgn_d = nc.dram_tensor("bgn", [128, 1], F32, kind="ExternalInput")
    bo_d = nc.dram_tensor("bo", [128, DOUT], F32, kind="ExternalInput")
    cos_d = nc.dram_tensor("cost", [128, N], BF16, kind="ExternalInput")
    sin_d = nc.dram_tensor("sint", [128, N], BF16, kind="ExternalInput")
    cosq_d = nc.dram_tensor("cosq", [128, R], BF16, kind="ExternalInput")
    sinq_d = nc.dram_tensor("sinq", [128, R], BF16, kind="ExternalInput")
    out_d = nc.dram_tensor("out", [R, DOUT], F32, kind="ExternalOutput")
    alb_d = nc.dram_tensor("alb", [1, 512], F32)
    gd_d = nc.dram_tensor("gd", [H, R], F32)

    # ---------------- SBUF ----------------
    xt_s = [nc.alloc_sbuf_tensor(f"xt{i}", [128, N], BF16) for i in range(DT)]
    xq_s = [nc.alloc_sbuf_tensor(f"xq{i}", [128, R], BF16) for i in range(DT)]
    wk_s = [[nc.alloc_sbuf_tensor(f"wk{j}_{i}", [128, 128], BF16) for i in range(DT)]
            for j in range(PAIRS)]
    wq_s = [[nc.alloc_sbuf_tensor(f"wq{j}_{i}", [128, 128], BF16) for i in range(DT)]
            for j in range(PAIRS)]
    wv_s = [nc.alloc_sbuf_tensor(f"wv{i}", [128, H * dh], BF16) for i in range(DT)]
    wg_s = [nc.alloc_sbuf_tensor(f"wg{i}", [128, H], BF16) for i in range(DT)]
    wo_s = [nc.alloc_sbuf_tensor(f"wo{j}", [128, DOUT], BF16) for j in range(PAIRS)]
    bgn_s = nc.alloc_sbuf_tensor("bgns", [128, 1], F32)
    bo_s = nc.alloc_sbuf_tensor("bos", [128, DOUT], F32)
    cos_s = nc.alloc_sbuf_tensor("coss", [128, N], BF16)
    sin_s = nc.alloc_sbuf_tensor("sins", [128, N], BF16)
    cosq_s = nc.alloc_sbuf_tensor("cosqs", [128, R], BF16)
    sinq_s = nc.alloc_sbuf_tensor("sinqs", [128, R], BF16)

    kn_s = [nc.alloc_sbuf_tensor(f"kn{j}", [128, N], BF16) for j in range(PAIRS)]
    ks_s = [nc.alloc_sbuf_tensor(f"ks{j}", [128, N], BF16) for j in range(min(2, PAIRS))]
    qn_s = [nc.alloc_sbuf_tensor(f"qn{j}", [128, R], BF16) for j in range(PAIRS)]
    qs_s = [nc.alloc_sbuf_tensor(f"qs{j}", [128, R], BF16) for j in range(min(2, PAIRS))]
    v_s = [nc.alloc_sbuf_tensor(f"v{t}", [128, H * 65], BF16) for t in range(KT)]

    QCHUNKS = _split(R, 512)          # (start, len) query chunks
    QCLEN = max(l for _, l in QCHUNKS)
    assert all(l == QCLEN for _, l in QCHUNKS), "uniform q chunks required"
    GROUPS = _split(KT, 3)            # (kt0, ngroup) keytile groups
    NG = len(GROUPS)

    p_sbuf = [nc.alloc_sbuf_tensor(f"p{i}", [128, QCLEN * 3], BF16) for i in range(2)]
    oT_s = [nc.alloc_sbuf_tensor(f"oT{j}", [128, QCLEN], BF16) for j in range(PAIRS)]
    gT_s = nc.alloc_sbuf_tensor("gT", [H, R], F32)
    al_s = nc.alloc_sbuf_tensor("al", [1, QCLEN], F32)
    al2_s = nc.alloc_sbuf_tensor("al2", [1, QCLEN], F32)
    alr_s = nc.alloc_sbuf_tensor("alr", [64, QCLEN], F32)
    outb_s = [nc.alloc_sbuf_tensor(f"outb{i}", [128, DOUT], F32) for i in range(2)]

    # ---------------- PSUM (8 banks) ----------------
    s_ps = [nc.alloc_psum_tensor(f"sps{i}", [128, QCLEN * 3], F32) for i in range(2)]
    o_ps = nc.alloc_psum_tensor("ops", [128, 512], F32)
    t_ps = nc.alloc_psum_tensor("tps", [128, 512], F32)
    o_b = [o_ps, t_ps]

    # proj-phase views of the S psum banks
    proj_ps = [s_ps[0].ap()[:, 0:512], s_ps[0].ap()[:, 512:1024]]
    g_ps = s_ps[1].ap()[0:H, 0:512]

    # ---------------- shared schedules ----------------
    proj_chunks = []                  # (kind, j/tt, col_start, col_len)
    for j in range(PAIRS):
        for c0, cl in _split(N, 512):
            proj_chunks.append(("K", j, c0, cl))
    for j in range(PAIRS):
        for c0, cl in _split(R, 512):
            proj_chunks.append(("Q", j, c0, cl))
    for tt in range(KT):
        proj_chunks.append(("V", tt, 0, H * dh))
    GCHUNKS = _split(R, 512)
    for gi, (c0, cl) in enumerate(GCHUNKS):
        proj_chunks.append(("G", gi, c0, cl))
    NPROJ = len(proj_chunks)
    NCOPY = NPROJ - len(GCHUNKS)       # dve_copy counts K/Q/V chunks only

    n_in_dma = DT + DT + PAIRS * DT * 2 + DT + DT + PAIRS + 6
    NROPE = PAIRS * 3 * 2
    NIDX = len(QCHUNKS) * H            # (qc, h) pairs

    sems = {}

    def S(name):
        return sems[name]

    import contextlib
    es = contextlib.ExitStack()
    for name in ["in_sem", "swap_sem", "outd_sem", "brd_sem", "pe_proj", "pe_s",
                 "pe_av", "pe_out", "act_s", "act_g", "dve_copy", "dve_rope",
                 "dve_al", "dve_ot", "dve_badd", "alb_sem", "g2_sem", "gd_sem", "dve_gfin"]:
        sems[name] = es.enter_context(nc.semaphore(name))

    with nc.Block() as block:

        # ================= SP: DMAs =================
        @block.sync
        def _(sp):
            def ld(dst, src):
                sp.dma_start(out=dst, in_=src).then_inc(S("in_sem"), 16)

            for i in range(DT):
                ld(xt_s[i].ap(), xt_d.ap()[i * 128:(i + 1) * 128, :])
            for j in range(PAIRS):
                for i in range(DT):
                    ld(wk_s[j][i].ap(), wk_d.ap()[j, i])
            for i in range(DT):
                ld(xq_s[i].ap(), xq_d.ap()[i * 128:(i + 1) * 128, :])
            for j in range(PAIRS):
                for i in range(DT):
                    ld(wq_s[j][i].ap(), wq_d.ap()[j, i])
            for i in range(DT):
                ld(wv_s[i].ap(), wv_d.ap()[i])
            for i in range(DT):
                ld(wg_s[i].ap(), wg_d.ap()[i])
            for j in range(PAIRS):
                ld(wo_s[j].ap(), wo_d.ap()[j])
            ld(bgn_s.ap(), bgn_d.ap())
            ld(bo_s.ap(), bo_d.ap())
            ld(cos_s.ap(), cos_d.ap())
            ld(sin_s.ap(), sin_d.ap())
            ld(cosq_s.ap(), cosq_d.ap())
            ld(sinq_s.ap(), sinq_d.ap())

            if phase < 2:
                return
            # swap copies for rope (partition-block swap via DMA)
            nk = PAIRS * len(_split(N, 512))
            nq = PAIRS * len(_split(R, 512))
            sp.wait_ge(S("dve_copy"), nk)
            nb = min(2, PAIRS)
            for j in range(PAIRS):
                if j >= nb:
                    sp.wait_ge(S("dve_rope"), 3 * (j - nb + 1))
                for a, b in ((0, 32), (32, 0), (64, 96), (96, 64)):
                    sp.dma_start(
                        out=ks_s[j % nb].ap()[b:b + 32, :],
                        in_=kn_s[j].ap()[a:a + 32, :],
                    ).then_inc(S("swap_sem"), 16)
            sp.wait_ge(S("dve_copy"), nk + nq)
            for j in range(PAIRS):
                if j >= nb:
                    sp.wait_ge(S("dve_rope"), 3 * PAIRS + 3 * (j - nb + 1))
                for a, b in ((0, 32), (32, 0), (64, 96), (96, 64)):
                    sp.dma_start(
                        out=qs_s[j % nb].ap()[b:b + 32, :],
                        in_=qn_s[j].ap()[a:a + 32, :],
                    ).then_inc(S("swap_sem"), 16)

            if phase < 5:
                return
            # gates to DRAM, alpha broadcasts via DRAM bounce, output stores
            sp.wait_ge(S("dve_gfin"), 1)
            sp.dma_start(out=gd_d.ap(), in_=gT_s.ap()).then_inc(S("gd_sem"), 16)
            sp.wait_ge(S("gd_sem"), 16)
            alb_ap = alb_d.ap()[:, 0:QCLEN]
            alr_bcast = bass.AP(
                tensor=alb_ap.tensor, offset=alb_ap.offset,
                ap=[[0, 64]] + alb_ap.ap[1:])
            for qci, (q0, qcl) in enumerate(QCHUNKS):
                for h in range(H):
                    idx = qci * H + h
                    sp.wait_ge(S("dve_al"), idx)         # al2 consumed
                    sp.dma_start(out=al2_s.ap()[:, 0:qcl],
                                 in_=gd_d.ap()[h:h + 1, q0:q0 + qcl]).then_inc(
                        S("g2_sem"), 16)
                    sp.wait_ge(S("dve_al"), idx + 1)
                    sp.dma_start(out=alb_ap, in_=al_s.ap()).then_inc(
                        S("alb_sem"), 16)
                    sp.wait_ge(S("alb_sem"), 16 * (idx + 1))
                    if idx >= 1:
                        sp.wait_ge(S("dve_ot"), idx)     # alr consumed
                    sp.dma_start(out=alr_s.ap(), in_=alr_bcast).then_inc(
                        S("brd_sem"), 16)
                if phase < 6:
                    continue
                for qt in range(qcl // 128):
                    gq = q0 // 128 + qt
                    sp.wait_ge(S("dve_badd"), gq + 1)
                    sp.dma_start(
                        out=out_d.ap()[gq * 128:(gq + 1) * 128, :],
                        in_=outb_s[gq % 2].ap(),
                    ).then_inc(S("outd_sem"), 16)

        # ================= PE =================
        @block.tensor
        def _(pe):
            need = {"K": 16 * (DT + PAIRS * DT),
                    "Q": 16 * (2 * DT + 2 * PAIRS * DT),
                    "V": 16 * (2 * DT + 2 * PAIRS * DT + DT),
                    "G": 16 * (2 * DT + 2 * PAIRS * DT + 2 * DT)}
            seen = set()

            # ---- projections ----
            gidx = 0
            for i, (kind, jt, c0, cl) in enumerate(proj_chunks):
                if kind not in seen:
                    seen.add(kind)
                    pe.wait_ge(S("in_sem"), need[kind])
                if kind == "G":
                    if gidx >= 1:
                        pe.wait_ge(S("act_g"), gidx)     # g_ps reused
                elif i >= 2:
                    pe.wait_ge(S("dve_copy"), min(i - 1, NCOPY))
                buf = proj_ps[i % 2]
                for dt in range(DT):
                    if kind == "K":
                        o, l, r = buf[:, 0:cl], wk_s[jt][dt].ap(), xt_s[dt].ap()[:, c0:c0 + cl]
                    elif kind == "Q":
                        o, l, r = buf[:, 0:cl], wq_s[jt][dt].ap(), xq_s[dt].ap()[:, c0:c0 + cl]
                    elif kind == "V":
                        o, l, r = buf[:, 0:cl], xt_s[dt].ap()[:, jt * 128:(jt + 1) * 128], wv_s[dt].ap()
                    else:  # G: gates^T [H, cl]
                        o, l, r = g_ps[:, 0:cl], wg_s[dt].ap(), xq_s[dt].ap()[:, c0:c0 + cl]
                    ins = pe.matmul(o, l, r, start=(dt == 0), stop=(dt == DT - 1))
                ins.then_inc(S("pe_proj"), 1)
                if kind == "G":
                    gidx += 1

            # ---- attention ----
            if phase < 3:
                return
            pe.wait_ge(S("in_sem"), 16 * n_in_dma)
            pe.wait_ge(S("dve_rope"), NROPE)
            pe.wait_ge(S("dve_copy"), NCOPY)
            if phase >= 2:
                pe.wait_ge(S("act_g"), len(GCHUNKS))   # g_ps bank reused by S groups

            Gg = 0
            for qci, (q0, qcl) in enumerate(QCHUNKS):
                for h in range(H):
                    idx = qci * H + h
                    j, par = h // 2, h % 2
                    base = 64 * par

                    def s_group(g):
                        kt0, ng = GROUPS[g]
                        gg = Gg + g
                        for l in range(ng):
                            kt = kt0 + l
                            ins = pe.matmul(
                                s_ps[gg % 2].ap()[:, qcl * l: qcl * (l + 1)],
                                kn_s[j].ap()[base:base + 64, kt * 128:(kt + 1) * 128],
                                qn_s[j].ap()[base:base + 64, q0:q0 + qcl],
                            )
                        ins.then_inc(S("pe_s"), 1)

                    s_group(0)
                    if NG > 1:
                        s_group(1)
                    for g in range(NG):
                        kt0, ng = GROUPS[g]
                        gg = Gg + g
                        pe.wait_ge(S("act_s"), gg + 1)
                        if g == 0 and phase >= 5 and idx >= 2:
                            pe.wait_ge(S("dve_ot"), idx - 1)   # this bank consumed
                        if g == 0 and phase >= 6 and idx % 2 == 1 and qci > 0:
                            pe.wait_ge(S("dve_badd"), qci * (qcl // 128))
                        last = None
                        if phase >= 4:
                            for l in range(ng):
                                kt = kt0 + l
                                last = pe.matmul(
                                    o_b[idx % 2].ap()[0:65, 0:qcl],
                                    v_s[kt].ap()[:, 65 * h: 65 * h + 65],
                                    p_sbuf[gg % 2].ap()[:, qcl * l: qcl * (l + 1)],
                                    start=(kt == 0), stop=(kt == KT - 1),
                                    skip_group_check=True,
                                )
                        if g + 2 < NG:
                            s_group(g + 2)
                        if phase >= 4 and g == NG - 1:
                            last.then_inc(S("pe_av"), 1)

                    Gg += NG

                # out-projection for this q chunk
                if phase < 6:
                    continue
                pe.wait_ge(S("dve_ot"), (qci + 1) * H)
                for qt in range(qcl // 128):
                    gq = q0 // 128 + qt
                    pe.wait_ge(S("dve_badd"), gq)        # t_ps bank free
                    for j2 in range(PAIRS):
                        ins = pe.matmul(
                            t_ps.ap()[:, 0:DOUT],
                            oT_s[j2].ap()[:, 128 * qt:128 * (qt + 1)],
                            wo_s[j2].ap(),
                            start=(j2 == 0), stop=(j2 == PAIRS - 1),
                        )
                    ins.then_inc(S("pe_out"), 1)

        # ================= ACT =================
        @block.scalar
        def _(act):
            if phase < 2:
                return
            # gates: u = exp(-(z + bg)) straight from PSUM, per-partition bias
            ng_chunk_base = NPROJ - len(GCHUNKS)
            act.wait_ge(S("in_sem"), 16 * n_in_dma)      # bgn
            for gi, (c0, cl) in enumerate(GCHUNKS):
                act.wait_ge(S("pe_proj"), ng_chunk_base + gi + 1)
                act.activation(
                    gT_s.ap()[:, c0:c0 + cl], g_ps[:, 0:cl], EXP,
                    bias=bgn_s.ap()[0:H, :], scale=-1.0,
                ).then_inc(S("act_g"), 1)

            if phase < 3:
                return
            for gg in range(NIDX * NG):
                g = gg % NG
                qci = gg // (NG * H)
                _, qcl = QCHUNKS[qci]
                _, ngrp = GROUPS[g]
                act.wait_ge(S("pe_s"), gg + 1)
                act.activation(
                    p_sbuf[gg % 2].ap()[:, 0:qcl * ngrp],
                    s_ps[gg % 2].ap()[:, 0:qcl * ngrp],
                    EXP, scale=0.125,
                ).then_inc(S("act_s"), 1)

        # ================= DVE =================
        @block.vector
        def _(dve):
            # proj copies (K/Q/V only; gates go straight PSUM->ACT)
            for i, (kind, jt, c0, cl) in enumerate(proj_chunks):
                if kind == "G":
                    continue
                dve.wait_ge(S("pe_proj"), i + 1)
                buf = proj_ps[i % 2]
                if kind == "K":
                    ins = dve.tensor_copy(kn_s[jt].ap()[:, c0:c0 + cl], buf[:, 0:cl])
                elif kind == "Q":
                    ins = dve.tensor_copy(qn_s[jt].ap()[:, c0:c0 + cl], buf[:, 0:cl])
                else:  # V
                    vt3 = v_s[jt].ap().rearrange("p (h c) -> p h c", c=65)
                    dve.memset(vt3[:, :, 64:65], 1.0)
                    for h in range(H):
                        ins = dve.tensor_copy(
                            vt3[:, h, 0:dh], buf[:, h * dh:(h + 1) * dh])
                ins.then_inc(S("dve_copy"), 1)

            if phase < 2:
                return
            # rope: x~ = norm*C + swap*S   (drain: same-engine RAW)
            dve.wait_ge(S("in_sem"), 16 * n_in_dma)      # cos/sin tables
            nb = min(2, PAIRS)
            for j in range(PAIRS):
                dve.wait_ge(S("swap_sem"), 16 * (j + 1) * 4)
                dve.tensor_mul(kn_s[j].ap(), kn_s[j].ap(), cos_s.ap()).then_inc(S("dve_rope"), 1)
                dve.tensor_mul(ks_s[j % nb].ap(), ks_s[j % nb].ap(), sin_s.ap()).then_inc(S("dve_rope"), 1)
                dve.drain()
                dve.tensor_add(kn_s[j].ap(), kn_s[j].ap(), ks_s[j % nb].ap()).then_inc(S("dve_rope"), 1)
            for j in range(PAIRS):
                dve.wait_ge(S("swap_sem"), 16 * (PAIRS * 4 + (j + 1) * 4))
                dve.tensor_mul(qn_s[j].ap(), qn_s[j].ap(), cosq_s.ap()).then_inc(S("dve_rope"), 1)
                dve.tensor_mul(qs_s[j % nb].ap(), qs_s[j % nb].ap(), sinq_s.ap()).then_inc(S("dve_rope"), 1)
                dve.drain()
                dve.tensor_add(qn_s[j].ap(), qn_s[j].ap(), qs_s[j % nb].ap()).then_inc(S("dve_rope"), 1)

            # gates finish: g = 1 / (1 + u)
            dve.wait_ge(S("act_g"), len(GCHUNKS))
            dve.tensor_scalar_add(gT_s.ap(), gT_s.ap(), 1.0)
            dve.drain()
            dve.reciprocal(gT_s.ap(), gT_s.ap()).then_inc(S("dve_gfin"), 1)

            # attention epilogues
            if phase < 5:
                return
            for qci, (q0, qcl) in enumerate(QCHUNKS):
                for h in range(H):
                    idx = qci * H + h
                    j, par = h // 2, h % 2
                    base = 64 * par
                    dve.wait_ge(S("pe_av"), idx + 1)
                    dve.reciprocal(al_s.ap(), o_b[idx % 2].ap()[64:65, 0:qcl])
                    dve.drain()
                    dve.wait_ge(S("g2_sem"), 16 * (idx + 1))
                    dve.tensor_mul(al_s.ap(), al_s.ap(),
                                   al2_s.ap()[:, 0:qcl]).then_inc(S("dve_al"), 1)
                    dve.wait_ge(S("brd_sem"), 16 * (idx + 1))
                    dve.tensor_mul(
                        oT_s[j].ap()[base:base + 64, 0:qcl],
                        o_b[idx % 2].ap()[0:64, 0:qcl],
                        alr_s.ap(),
                    ).then_inc(S("dve_ot"), 1)

                if phase < 6:
                    continue
                for qt in range(qcl // 128):
                    gq = q0 // 128 + qt
                    dve.wait_ge(S("pe_out"), gq + 1)
                    if gq >= 2:
                        dve.wait_ge(S("outd_sem"), 16 * (gq - 1))
                    dve.tensor_add(
                        outb_s[gq % 2].ap(), t_ps.ap()[:, 0:DOUT], bo_s.ap(),
                    ).then_inc(S("dve_badd"), 1)

    es.close()
    return nc


# =====================================================================
# Host side
# =====================================================================

_CACHED = {}


def _rope_tables(n):
    half = DIM_HEAD // 2                      # 32
    inv_freq = 1.0 / (MAX_FREQ ** (np.arange(0, half, 2, dtype=np.float32) / half))
    invf = np.concatenate([inv_freq, inv_freq])            # [32]
    t = np.arange(n, dtype=np.float32)
    ang = invf[:, None] * t[None, :]                       # [32, n]
    return np.cos(ang), np.sin(ang)


def _prep_shared(wq_full, wk_full, wv_full, wg_full, bg_full, wo_full, bo_full,
                 N, H=HEADS):
    dh = DIM_HEAD
    D = wq_full.shape[0]
    DT = D // 128
    PAIRS = H // 2
    perm = np.concatenate([np.arange(0, dh, 2), np.arange(1, dh, 2)])
    bf = ml_dtypes.bfloat16

    def pack_pairs(w):
        arr = np.zeros((PAIRS, DT, 128, 128), np.float32)
        for j in range(PAIRS):
            pair = np.concatenate(
                [w[:, 64 * (2 * j + p):64 * (2 * j + p) + 64][:, perm]
                 for p in (0, 1)], axis=1)               # [D, 128]
            for dt in range(DT):
                arr[j, dt] = pair[128 * dt:128 * (dt + 1), :]
        return arr.astype(bf)

    wk = pack_pairs(wk_full)
    wq = pack_pairs(wq_full)
    wv = np.stack([wv_full[128 * dt:128 * (dt + 1), :] for dt in range(DT)]).astype(bf)
    wg = np.stack([wg_full[128 * dt:128 * (dt + 1), :] for dt in range(DT)]).astype(bf)
    wo = np.stack([wo_full[128 * j:128 * (j + 1), :] for j in range(PAIRS)]).astype(bf)
    bgn = np.zeros((128, 1), np.float32)
    bgn[:H, 0] = -bg_full
    bo = np.tile(bo_full[None, :], (128, 1)).astype(np.float32)

    cos32, sin32 = _rope_tables(N)                         # [32, N]
    cost = np.tile(cos32, (4, 1)).astype(bf)               # [128, N]
    sint = np.tile(np.concatenate([-sin32, sin32], 0), (2, 1)).astype(bf)
    return dict(wk=wk, wq=wq, wv=wv, wg=wg, wo=wo, bgn=bgn, bo=bo,
                cost=cost, sint=sint)


def kernel(x, w_qkv, w_gates, b_gates, w_out, b_out):
    b, n, d = x.shape
    R = (b * n) // N_CORES
    bf = ml_dtypes.bfloat16

    key = ("nc", d, n, R)
    if key not in _CACHED:
        _CACHED[key] = build_nc(D=d, N=n, R=R, H=HEADS, DOUT=d)
    nc = _CACHED[key]

    w_qkv = np.asarray(w_qkv, np.float32)
    inner = HEADS * DIM_HEAD
    shared = _prep_shared(
        w_qkv[:, 0:inner], w_qkv[:, inner:2 * inner], w_qkv[:, 2 * inner:3 * inner],
        np.asarray(w_gates, np.float32), np.asarray(b_gates, np.float32),
        np.asarray(w_out, np.float32), np.asarray(b_out, np.float32), n)

    per_batch = N_CORES // b
    in_maps = []
    for c in range(N_CORES):
        bi = c // per_batch
        q0 = (c % per_batch) * R
        xt = np.ascontiguousarray(np.asarray(x[bi], np.float32).T).astype(bf)
        xq = np.ascontiguousarray(xt[:, q0:q0 + R])
        in_maps.append(dict(
            xt=xt, xq=xq,
            cosq=np.ascontiguousarray(shared["cost"][:, q0:q0 + R]),
            sinq=np.ascontiguousarray(shared["sint"][:, q0:q0 + R]),
            **shared))

    res = run_bass_kernel_spmd(nc, in_maps, core_ids=list(range(N_CORES)))
    out = np.concatenate([res.results[c]["out"] for c in range(N_CORES)], axis=0)
    return out.reshape(b, n, d).astype(np.float32)


# revision 32
# speedup vs baseline: 1.7396x; 1.3768x over previous
"""Trainium2 Bass kernel for nn_Attention_61177514164290.

Gated multi-head attention with RoPE:
  qkv = x @ w_qkv ; rope(q), rope(k) ; attn = softmax(q k^T / 8)
  out = (attn @ v) * sigmoid(x @ w_gates + b_gates) ; out @ w_out + b_out

Sharding: row-parallel over (batch, query-rows). 8 cores, core c owns batch
c//4 and query rows [(c%4)*1024, +1024) for ALL 8 heads. K/V projections are
recomputed per core for its batch (cheaper than any inter-core collective on
this chip), so there are ZERO collectives; the host concatenates the 8 row
slices.

Per-core dataflow (all matmuls bf16 with f32 PSUM accumulation):
  - x^T tiles [128d, N] -> K^T/Q^T projections with host-permuted weights
    (even dh dims then odd) so RoPE pairs become 32-row partition blocks.
  - RoPE = norm .* C + swap .* S on DVE, where `swap` is a partition-block
    swapped copy made by SBUF->SBUF DMA and S has the -sin/+sin signs baked.
  - S^T tile [128 keys, 512 q] = (K~ pair slice [64,128]).T @ Q~ [64, 512];
    exp on the Scalar engine reads 3 PSUM banks per instruction (scale=1/8,
    no max-subtraction needed: |s|/8 < ~6), writes P~ bf16 to SBUF.
  - O^T [65, 512] accumulates ([V_h | 1]).T @ P~ over 32 keytiles in a single
    full-bank PSUM region (start=True resets accumulation state bank-wide,
    so per-qtile regions must not interleave); row 64 = softmax denominators.
  - alpha row = gate^T[h] / denominators (DVE), DMA-broadcast across 64
    partitions, one DVE multiply scales O^T into the out-projection layout.
  - out-projection packs head-pairs to K=128, + b_out, DMA out.
"""

import sys

for _p in ("/opt/trn_rl_repo",):
    if _p not in sys.path:
        sys.path.insert(0, _p)

import numpy as np
import ml_dtypes

import concourse.bass as bass
from concourse import mybir
from concourse.bass_utils import run_bass_kernel_spmd

BF16 = mybir.dt.bfloat16
F32 = mybir.dt.float32
EXP = mybir.ActivationFunctionType.Exp

N_CORES = 8
HEADS = 8
DIM_HEAD = 64
MAX_FREQ = 10.0


def _split(n, size):
    out = []
    i = 0
    while i < n:
        out.append((i, min(size, n - i)))
        i += size
    return out


def build_nc(D=512, N=4096, R=1024, H=8, DOUT=512, phase=6):
    DT = D // 128          # d-tiles
    KT = N // 128          # key tiles
    PAIRS = H // 2
    dh = DIM_HEAD

    nc = bass.Bass()

    # ---------------- DRAM parameters ----------------
    xt_d = nc.dram_tensor("xt", [D, N], BF16, kind="ExternalInput")
    xq_d = nc.dram_tensor("xq", [D, R], BF16, kind="ExternalInput")
    wk_d = nc.dram_tensor("wk", [PAIRS, DT, 128, 128], BF16, kind="ExternalInput")
    wq_d = nc.dram_tensor("wq", [PAIRS, DT, 128, 128], BF16, kind="ExternalInput")
    wv_d = nc.dram_tensor("wv", [DT, 128, H * dh], BF16, kind="ExternalInput")
    wg_d = nc.dram_tensor("wg", [DT, 128, H], BF16, kind="ExternalInput")
    wo_d = nc.dram_tensor("wo", [PAIRS, 128, DOUT], BF16, kind="ExternalInput")
    bgn_d = nc.dram_tensor("bgn", [128, 1], F32, kind="ExternalInput")
    bo_d = nc.dram_tensor("bo", [128, DOUT], F32, kind="ExternalInput")
    cos_d = nc.dram_tensor("cost", [128, N], BF16, kind="ExternalInput")
    sin_d = nc.dram_tensor("sint", [128, N], BF16, kind="ExternalInput")
    cosq_d = nc.dram_tensor("cosq", [128, R], BF16, kind="ExternalInput")
    sinq_d = nc.dram_tensor("sinq", [128, R], BF16, kind="ExternalInput")
    out_d = nc.dram_tensor("out", [R, DOUT], F32, kind="ExternalOutput")
    alb_d = nc.dram_tensor("alb", [1, 512], F32)
    gd_d = nc.dram_tensor("gd", [H, R], F32)

    # ---------------- SBUF ----------------
    xt_s = [nc.alloc_sbuf_tensor(f"xt{i}", [128, N], BF16) for i in range(DT)]
    xq_s = [nc.alloc_sbuf_tensor(f"xq{i}", [128, R], BF16) for i in range(DT)]
    wk_s = [[nc.alloc_sbuf_tensor(f"wk{j}_{i}", [128, 128], BF16) for i in range(DT)]
            for j in range(PAIRS)]
    wq_s = [[nc.alloc_sbuf_tensor(f"wq{j}_{i}", [128, 128], BF16) for i in range(DT)]
            for j in range(PAIRS)]
    wv_s = [nc.alloc_sbuf_tensor(f"wv{i}", [128, H * dh], BF16) for i in range(DT)]
    wg_s = [nc.alloc_sbuf_tensor(f"wg{i}", [128, H], BF16) for i in range(DT)]
    wo_s = [nc.alloc_sbuf_tensor(f"wo{j}", [128, DOUT], BF16) for j in range(PAIRS)]
    bgn_s = nc.alloc_sbuf_tensor("bgns", [128, 1], F32)
    bo_s = nc.alloc_sbuf_tensor("bos", [128, DOUT], F32)
    cos_s = nc.alloc_sbuf_tensor("coss", [128, N], BF16)
    sin_s = nc.alloc_sbuf_tensor("sins", [128, N], BF16)
    cosq_s = nc.alloc_sbuf_tensor("cosqs", [128, R], BF16)
    sinq_s = nc.alloc_sbuf_tensor("sinqs", [128, R], BF16)

    kn_s = [nc.alloc_sbuf_tensor(f"kn{j}", [128, N], BF16) for j in range(PAIRS)]
    ks_s = [nc.alloc_sbuf_tensor(f"ks{j}", [128, N], BF16) for j in range(min(2, PAIRS))]
    qn_s = [nc.alloc_sbuf_tensor(f"qn{j}", [128, R], BF16) for j in range(PAIRS)]
    qs_s = [nc.alloc_sbuf_tensor(f"qs{j}", [128, R], BF16) for j in range(min(2, PAIRS))]
    v_s = [nc.alloc_sbuf_tensor(f"v{t}", [128, H * 65], BF16) for t in range(KT)]
    assert PAIRS <= DT
    kd_s = [nc.alloc_sbuf_tensor_at(f"kd{j}", [128, N], BF16,
                                    offset=nc.lookup_mloc(xt_s[j]).addr)
            for j in range(PAIRS)]
    qd_s = [nc.alloc_sbuf_tensor_at(f"qd{j}", [128, R], BF16,
                                    offset=nc.lookup_mloc(xq_s[j]).addr)
            for j in range(PAIRS)]

    QCHUNKS = _split(R, 512)          # (start, len) query chunks
    QCLEN = max(l for _, l in QCHUNKS)
    assert all(l == QCLEN for _, l in QCHUNKS), "uniform q chunks required"
    GROUPS = _split(KT, 3)            # (kt0, ngroup) keytile groups
    NG = len(GROUPS)

    p_sbuf = [nc.alloc_sbuf_tensor(f"p{i}", [128, QCLEN * 3], BF16) for i in range(2)]
    oT_s = [nc.alloc_sbuf_tensor(f"oT{j}", [128, QCLEN], BF16) for j in range(PAIRS)]
    gT_s = nc.alloc_sbuf_tensor("gT", [H, R], F32)
    al_s = nc.alloc_sbuf_tensor("al", [1, QCLEN], F32)
    al2_s = nc.alloc_sbuf_tensor("al2", [1, QCLEN], F32)
    alr_s = nc.alloc_sbuf_tensor("alr", [64, QCLEN], F32)
    outb_s = [nc.alloc_sbuf_tensor(f"outb{i}", [128, DOUT], F32) for i in range(2)]

    # ---------------- PSUM (8 banks) ----------------
    s_ps = [nc.alloc_psum_tensor(f"sps{i}", [128, QCLEN * 3], F32) for i in range(2)]
    o_ps = nc.alloc_psum_tensor("ops", [128, 512], F32)
    t_ps = nc.alloc_psum_tensor("tps", [128, 512], F32)
    o_b = [o_ps, t_ps]

    # proj-phase views of the S psum banks
    proj_ps = [s_ps[0].ap()[:, 0:512], s_ps[0].ap()[:, 512:1024]]
    g_ps = s_ps[1].ap()[0:H, 0:512]

    # ---------------- shared schedules ----------------
    proj_chunks = []                  # (kind, j/tt, col_start, col_len)
    NKC = len(_split(N, 512))         # K chunks per pair
    NQC = len(_split(R, 512))
    for j in range(PAIRS):
        for c0, cl in _split(N, 512):
            proj_chunks.append(("K", j, c0, cl))
        for c0, cl in _split(R, 512):
            proj_chunks.append(("Q", j, c0, cl))
    for tt in range(KT):
        proj_chunks.append(("V", tt, 0, H * dh))
    GCHUNKS = _split(R, 512)
    for gi, (c0, cl) in enumerate(GCHUNKS):
        proj_chunks.append(("G", gi, c0, cl))
    NPROJ = len(proj_chunks)
    NCOPY = NPROJ - len(GCHUNKS)       # dve_copy counts K/Q/V chunks only

    n_in_dma = DT + DT + PAIRS * DT * 2 + DT + DT + PAIRS + 6
    NROPE = PAIRS * 3 * 2
    NIDX = len(QCHUNKS) * H            # (qc, h) pairs

    sems = {}

    def S(name):
        return sems[name]

    import contextlib
    es = contextlib.ExitStack()
    for name in ["in_sem", "swap_sem", "outd_sem", "brd_sem", "pe_proj", "pe_s",
                 "pe_av", "pe_out", "act_s", "act_g", "dve_copy", "dve_rope",
                 "dve_al", "dve_ot", "dve_badd", "alb_sem", "g2_sem", "gd_sem", "dve_gfin", "dup_sem"]:
        sems[name] = es.enter_context(nc.semaphore(name))

    with nc.Block() as block:

        # ================= SP: DMAs =================
        @block.sync
        def _(sp):
            def ld(dst, src):
                sp.dma_start(out=dst, in_=src).then_inc(S("in_sem"), 16)

            for i in range(DT):
                ld(xt_s[i].ap(), xt_d.ap()[i * 128:(i + 1) * 128, :])
            for j in range(PAIRS):
                for i in range(DT):
                    ld(wk_s[j][i].ap(), wk_d.ap()[j, i])
            for i in range(DT):
                ld(xq_s[i].ap(), xq_d.ap()[i * 128:(i + 1) * 128, :])
            for j in range(PAIRS):
                for i in range(DT):
                    ld(wq_s[j][i].ap(), wq_d.ap()[j, i])
            for i in range(DT):
                ld(wv_s[i].ap(), wv_d.ap()[i])
            for i in range(DT):
                ld(wg_s[i].ap(), wg_d.ap()[i])
            for j in range(PAIRS):
                ld(wo_s[j].ap(), wo_d.ap()[j])
            ld(bgn_s.ap(), bgn_d.ap())
            ld(bo_s.ap(), bo_d.ap())
            ld(cos_s.ap(), cos_d.ap())
            ld(sin_s.ap(), sin_d.ap())
            ld(cosq_s.ap(), cosq_d.ap())
            ld(sinq_s.ap(), sinq_d.ap())

            if phase < 2:
                return
            # swap copies for rope (partition-block swap via DMA), K/Q per pair
            nb = min(2, PAIRS)
            for j in range(PAIRS):
                sp.wait_ge(S("dve_copy"), (NKC + NQC) * j + NKC)
                if j >= nb:
                    sp.wait_ge(S("dve_rope"), 6 * (j - nb) + 3)
                for a, b in ((0, 32), (32, 0), (64, 96), (96, 64)):
                    sp.dma_start(
                        out=ks_s[j % nb].ap()[b:b + 32, :],
                        in_=kn_s[j].ap()[a:a + 32, :],
                    ).then_inc(S("swap_sem"), 16)
                sp.wait_ge(S("dve_copy"), (NKC + NQC) * (j + 1))
                if j >= nb:
                    sp.wait_ge(S("dve_rope"), 6 * (j - nb) + 6)
                for a, b in ((0, 32), (32, 0), (64, 96), (96, 64)):
                    sp.dma_start(
                        out=qs_s[j % nb].ap()[b:b + 32, :],
                        in_=qn_s[j].ap()[a:a + 32, :],
                    ).then_inc(S("swap_sem"), 16)

            if phase >= 3:
                # duplicate roped K~/Q~ with swapped halves into dead xt/xq space
                sp.wait_ge(S("pe_proj"), NPROJ)
                for j in range(PAIRS):
                    sp.wait_ge(S("dve_rope"), 6 * (j + 1))
                    for a, b in ((0, 64), (64, 0)):
                        sp.dma_start(out=kd_s[j].ap()[b:b + 64, :],
                                     in_=kn_s[j].ap()[a:a + 64, :]).then_inc(S("dup_sem"), 16)
                        sp.dma_start(out=qd_s[j].ap()[b:b + 64, :],
                                     in_=qn_s[j].ap()[a:a + 64, :]).then_inc(S("dup_sem"), 16)

            if phase < 5:
                return
            # gates to DRAM, alpha broadcasts via DRAM bounce, output stores
            sp.wait_ge(S("dve_gfin"), 1)
            sp.dma_start(out=gd_d.ap(), in_=gT_s.ap()).then_inc(S("gd_sem"), 16)
            sp.wait_ge(S("gd_sem"), 16)
            alb_ap = alb_d.ap()[:, 0:QCLEN]
            alr_bcast = bass.AP(
                tensor=alb_ap.tensor, offset=alb_ap.offset,
                ap=[[0, 64]] + alb_ap.ap[1:])
            for qci, (q0, qcl) in enumerate(QCHUNKS):
                for h in range(H):
                    idx = qci * H + h
                    sp.wait_ge(S("dve_al"), idx)         # al2 consumed
                    sp.dma_start(out=al2_s.ap()[:, 0:qcl],
                                 in_=gd_d.ap()[h:h + 1, q0:q0 + qcl]).then_inc(
                        S("g2_sem"), 16)
                    sp.wait_ge(S("dve_al"), idx + 1)
                    sp.dma_start(out=alb_ap, in_=al_s.ap()).then_inc(
                        S("alb_sem"), 16)
                    sp.wait_ge(S("alb_sem"), 16 * (idx + 1))
                    if idx >= 1:
                        sp.wait_ge(S("dve_ot"), idx)     # alr consumed
                    sp.dma_start(out=alr_s.ap(), in_=alr_bcast).then_inc(
                        S("brd_sem"), 16)
                if phase < 6:
                    continue
                for qt in range(qcl // 128):
                    gq = q0 // 128 + qt
                    sp.wait_ge(S("dve_badd"), gq + 1)
                    sp.dma_start(
                        out=out_d.ap()[gq * 128:(gq + 1) * 128, :],
                        in_=outb_s[gq % 2].ap(),
                    ).then_inc(S("outd_sem"), 16)

        # ================= PE =================
        @block.tensor
        def _(pe):
            need = {"K": 16 * (DT + PAIRS * DT),
                    "Q": 16 * (2 * DT + 2 * PAIRS * DT),
                    "V": 16 * (2 * DT + 2 * PAIRS * DT + DT),
                    "G": 16 * (2 * DT + 2 * PAIRS * DT + 2 * DT)}
            seen = set()

            # ---- projections ----
            gidx = 0
            for i, (kind, jt, c0, cl) in enumerate(proj_chunks):
                if kind not in seen:
                    seen.add(kind)
                    pe.wait_ge(S("in_sem"), need[kind])
                if kind == "G":
                    if gidx >= 1:
                        pe.wait_ge(S("act_g"), gidx)     # g_ps reused
                elif i >= 2:
                    pe.wait_ge(S("dve_copy"), min(i - 1, NCOPY))
                buf = proj_ps[i % 2]
                for dt in range(DT):
                    if kind == "K":
                        o, l, r = buf[:, 0:cl], wk_s[jt][dt].ap(), xt_s[dt].ap()[:, c0:c0 + cl]
                    elif kind == "Q":
                        o, l, r = buf[:, 0:cl], wq_s[jt][dt].ap(), xq_s[dt].ap()[:, c0:c0 + cl]
                    elif kind == "V":
                        o, l, r = buf[:, 0:cl], xt_s[dt].ap()[:, jt * 128:(jt + 1) * 128], wv_s[dt].ap()
                    else:  # G: gates^T [H, cl]
                        o, l, r = g_ps[:, 0:cl], wg_s[dt].ap(), xq_s[dt].ap()[:, c0:c0 + cl]
                    ins = pe.matmul(o, l, r, start=(dt == 0), stop=(dt == DT - 1))
                ins.then_inc(S("pe_proj"), 1)
                if kind == "G":
                    gidx += 1

            # ---- attention ----
            if phase < 3:
                return
            pe.wait_ge(S("in_sem"), 16 * n_in_dma)
            if phase >= 2:
                pe.wait_ge(S("act_g"), len(GCHUNKS))   # g_ps bank reused by S groups

            def outproj(qci2):
                q02, qcl2 = QCHUNKS[qci2]
                pe.wait_ge(S("dve_ot"), (qci2 + 1) * H)
                for qt in range(qcl2 // 128):
                    gq = q02 // 128 + qt
                    pe.wait_ge(S("dve_badd"), gq)        # t_ps bank free
                    for j2 in range(PAIRS):
                        ins = pe.matmul(
                            t_ps.ap()[:, 0:DOUT],
                            oT_s[j2].ap()[:, 128 * qt:128 * (qt + 1)],
                            wo_s[j2].ap(),
                            start=(j2 == 0), stop=(j2 == PAIRS - 1),
                        )
                    ins.then_inc(S("pe_out"), 1)

            Gg = 0
            s_cnt = 0
            for qci, (q0, qcl) in enumerate(QCHUNKS):
                for h in range(H):
                    idx = qci * H + h
                    j, par = h // 2, h % 2

                    if h == 2 * (h // 2) and qci == 0:
                        # pair j becomes active: rope + duplicates ready
                        pe.wait_ge(S("dve_rope"), 6 * (j + 1))
                        if phase >= 3:
                            pe.wait_ge(S("dup_sem"), 16 * 4 * (j + 1))

                    def s_group(g):
                        nonlocal s_cnt
                        kt0, ng = GROUPS[g]
                        gg = Gg + g
                        for l in range(ng):
                            kt = kt0 + l
                            b = 64 * (s_cnt % 2)
                            if b == 64 * par:
                                kk, qq = kn_s[j], qn_s[j]
                            else:
                                kk, qq = kd_s[j], qd_s[j]
                            ins = pe.matmul(
                                s_ps[gg % 2].ap()[:, qcl * l: qcl * (l + 1)],
                                kk.ap()[b:b + 64, kt * 128:(kt + 1) * 128],
                                qq.ap()[b:b + 64, q0:q0 + qcl],
                            )
                            s_cnt += 1
                        ins.then_inc(S("pe_s"), 1)

                    s_group(0)
                    if NG > 1:
                        s_group(1)
                    for g in range(NG):
                        kt0, ng = GROUPS[g]
                        gg = Gg + g
                        pe.wait_ge(S("act_s"), gg + 1)
                        if idx < 2:
                            pe.wait_ge(S("dve_copy"),
                                       PAIRS * (NKC + NQC) + min(kt0 + ng, KT))
                        if g == 0 and phase >= 5 and idx >= 2:
                            pe.wait_ge(S("dve_ot"), idx - 1)   # this bank consumed
                        if g == 0 and phase >= 6 and idx % 2 == 1 and qci > 0:
                            pe.wait_ge(S("dve_badd"), qci * (qcl // 128))
                        last = None
                        if phase >= 4:
                            for l in range(ng):
                                kt = kt0 + l
                                last = pe.matmul(
                                    o_b[idx % 2].ap()[0:65, 0:qcl],
                                    v_s[kt].ap()[:, 65 * h: 65 * h + 65],
                                    p_sbuf[gg % 2].ap()[:, qcl * l: qcl * (l + 1)],
                                    start=(kt == 0), stop=(kt == KT - 1),
                                    skip_group_check=True,
                                )
                        if g + 2 < NG:
                            s_group(g + 2)
                        if phase >= 4 and g == NG - 1:
                            last.then_inc(S("pe_av"), 1)

                    Gg += NG

                    # delayed out-projection of the previous q chunk
                    if phase >= 6 and h == 0 and qci > 0:
                        outproj(qci - 1)

            if phase >= 6:
                outproj(len(QCHUNKS) - 1)

        # ================= ACT =================
        @block.scalar
        def _(act):
            if phase < 2:
                return
            # gates: u = exp(-(z + bg)) straight from PSUM, per-partition bias
            ng_chunk_base = NPROJ - len(GCHUNKS)
            act.wait_ge(S("in_sem"), 16 * n_in_dma)      # bgn
            for gi, (c0, cl) in enumerate(GCHUNKS):
                act.wait_ge(S("pe_proj"), ng_chunk_base + gi + 1)
                act.activation(
                    gT_s.ap()[:, c0:c0 + cl], g_ps[:, 0:cl], EXP,
                    bias=bgn_s.ap()[0:H, :], scale=-1.0,
                ).then_inc(S("act_g"), 1)

            if phase < 3:
                return
            for gg in range(NIDX * NG):
                g = gg % NG
                qci = gg // (NG * H)
                _, qcl = QCHUNKS[qci]
                _, ngrp = GROUPS[g]
                act.wait_ge(S("pe_s"), gg + 1)
                act.activation(
                    p_sbuf[gg % 2].ap()[:, 0:qcl * ngrp],
                    s_ps[gg % 2].ap()[:, 0:qcl * ngrp],
                    EXP, scale=0.125,
                ).then_inc(S("act_s"), 1)

        # ================= DVE =================
        @block.vector
        def _(dve):
            # proj copies (K/Q/V only; gates go straight PSUM->ACT)
            for i, (kind, jt, c0, cl) in enumerate(proj_chunks):
                if kind == "G":
                    continue
                dve.wait_ge(S("pe_proj"), i + 1)
                buf = proj_ps[i % 2]
                if kind == "K":
                    ins = dve.tensor_copy(kn_s[jt].ap()[:, c0:c0 + cl], buf[:, 0:cl])
                elif kind == "Q":
                    ins = dve.tensor_copy(qn_s[jt].ap()[:, c0:c0 + cl], buf[:, 0:cl])
                else:  # V
                    vt3 = v_s[jt].ap().rearrange("p (h c) -> p h c", c=65)
                    dve.memset(vt3[:, :, 64:65], 1.0)
                    for h in range(H):
                        ins = dve.tensor_copy(
                            vt3[:, h, 0:dh], buf[:, h * dh:(h + 1) * dh])
                ins.then_inc(S("dve_copy"), 1)

            if phase < 2:
                return
            # rope: x~ = norm*C + swap*S   (drain: same-engine RAW)
            dve.wait_ge(S("in_sem"), 16 * n_in_dma)      # cos/sin tables
            nb = min(2, PAIRS)
            for j in range(PAIRS):
                dve.wait_ge(S("swap_sem"), 16 * (8 * j + 4))
                dve.tensor_mul(kn_s[j].ap(), kn_s[j].ap(), cos_s.ap()).then_inc(S("dve_rope"), 1)
                dve.tensor_mul(ks_s[j % nb].ap(), ks_s[j % nb].ap(), sin_s.ap()).then_inc(S("dve_rope"), 1)
                dve.drain()
                dve.tensor_add(kn_s[j].ap(), kn_s[j].ap(), ks_s[j % nb].ap()).then_inc(S("dve_rope"), 1)
                dve.wait_ge(S("swap_sem"), 16 * (8 * j + 8))
                dve.tensor_mul(qn_s[j].ap(), qn_s[j].ap(), cosq_s.ap()).then_inc(S("dve_rope"), 1)
                dve.tensor_mul(qs_s[j % nb].ap(), qs_s[j % nb].ap(), sinq_s.ap()).then_inc(S("dve_rope"), 1)
                dve.drain()
                dve.tensor_add(qn_s[j].ap(), qn_s[j].ap(), qs_s[j % nb].ap()).then_inc(S("dve_rope"), 1)

            # gates finish: g = 1 / (1 + u)
            dve.wait_ge(S("act_g"), len(GCHUNKS))
            dve.tensor_scalar_add(gT_s.ap(), gT_s.ap(), 1.0)
            dve.drain()
            dve.reciprocal(gT_s.ap(), gT_s.ap()).then_inc(S("dve_gfin"), 1)

            # attention epilogues
            if phase < 5:
                return
            for qci, (q0, qcl) in enumerate(QCHUNKS):
                for h in range(H):
                    idx = qci * H + h
                    j, par = h // 2, h % 2
                    base = 64 * par
                    dve.wait_ge(S("pe_av"), idx + 1)
                    dve.reciprocal(al_s.ap(), o_b[idx % 2].ap()[64:65, 0:qcl])
                    dve.drain()
                    dve.wait_ge(S("g2_sem"), 16 * (idx + 1))
                    dve.tensor_mul(al_s.ap(), al_s.ap(),
                                   al2_s.ap()[:, 0:qcl]).then_inc(S("dve_al"), 1)
                    dve.wait_ge(S("brd_sem"), 16 * (idx + 1))
                    dve.tensor_mul(
                        oT_s[j].ap()[base:base + 64, 0:qcl],
                        o_b[idx % 2].ap()[0:64, 0:qcl],
                        alr_s.ap(),
                    ).then_inc(S("dve_ot"), 1)

                if phase < 6:
                    continue
                for qt in range(qcl // 128):
                    gq = q0 // 128 + qt
                    dve.wait_ge(S("pe_out"), gq + 1)
                    if gq >= 2:
                        dve.wait_ge(S("outd_sem"), 16 * (gq - 1))
                    dve.tensor_add(
                        outb_s[gq % 2].ap(), t_ps.ap()[:, 0:DOUT], bo_s.ap(),
                    ).then_inc(S("dve_badd"), 1)

    es.close()
    return nc


# =====================================================================
# Host side
# =====================================================================

_CACHED = {}


def _rope_tables(n):
    half = DIM_HEAD // 2                      # 32
    inv_freq = 1.0 / (MAX_FREQ ** (np.arange(0, half, 2, dtype=np.float32) / half))
    invf = np.concatenate([inv_freq, inv_freq])            # [32]
    t = np.arange(n, dtype=np.float32)
    ang = invf[:, None] * t[None, :]                       # [32, n]
    return np.cos(ang), np.sin(ang)


def _prep_shared(wq_full, wk_full, wv_full, wg_full, bg_full, wo_full, bo_full,
                 N, H=HEADS):
    dh = DIM_HEAD
    D = wq_full.shape[0]
    DT = D // 128
    PAIRS = H // 2
    perm = np.concatenate([np.arange(0, dh, 2), np.arange(1, dh, 2)])
    bf = ml_dtypes.bfloat16

    def pack_pairs(w):
        arr = np.zeros((PAIRS, DT, 128, 128), np.float32)
        for j in range(PAIRS):
            pair = np.concatenate(
                [w[:, 64 * (2 * j + p):64 * (2 * j + p) + 64][:, perm]
                 for p in (0, 1)], axis=1)               # [D, 128]
            for dt in range(DT):
                arr[j, dt] = pair[128 * dt:128 * (dt + 1), :]
        return arr.astype(bf)

    wk = pack_pairs(wk_full)
    wq = pack_pairs(wq_full)
    wv = np.stack([wv_full[128 * dt:128 * (dt + 1), :] for dt in range(DT)]).astype(bf)
    wg = np.stack([wg_full[128 * dt:128 * (dt + 1), :] for dt in range(DT)]).astype(bf)
    wo = np.stack([wo_full[128 * j:128 * (j + 1), :] for j in range(PAIRS)]).astype(bf)
    bgn = np.zeros((128, 1), np.float32)
    bgn[:H, 0] = -bg_full
    bo = np.tile(bo_full[None, :], (128, 1)).astype(np.float32)

    cos32, sin32 = _rope_tables(N)                         # [32, N]
    cost = np.tile(cos32, (4, 1)).astype(bf)               # [128, N]
    sint = np.tile(np.concatenate([-sin32, sin32], 0), (2, 1)).astype(bf)
    return dict(wk=wk, wq=wq, wv=wv, wg=wg, wo=wo, bgn=bgn, bo=bo,
                cost=cost, sint=sint)


def kernel(x, w_qkv, w_gates, b_gates, w_out, b_out):
    b, n, d = x.shape
    R = (b * n) // N_CORES
    bf = ml_dtypes.bfloat16

    key = ("nc", d, n, R)
    if key not in _CACHED:
        _CACHED[key] = build_nc(D=d, N=n, R=R, H=HEADS, DOUT=d)
    nc = _CACHED[key]

    w_qkv = np.asarray(w_qkv, np.float32)
    inner = HEADS * DIM_HEAD
    shared = _prep_shared(
        w_qkv[:, 0:inner], w_qkv[:, inner:2 * inner], w_qkv[:, 2 * inner:3 * inner],
        np.asarray(w_gates, np.float32), np.asarray(b_gates, np.float32),
        np.asarray(w_out, np.float32), np.asarray(b_out, np.float32), n)

    per_batch = N_CORES // b
    in_maps = []
    for c in range(N_CORES):
        bi = c // per_batch
        q0 = (c % per_batch) * R
        xt = np.ascontiguousarray(np.asarray(x[bi], np.float32).T).astype(bf)
        xq = np.ascontiguousarray(xt[:, q0:q0 + R])
        in_maps.append(dict(
            xt=xt, xq=xq,
            cosq=np.ascontiguousarray(shared["cost"][:, q0:q0 + R]),
            sinq=np.ascontiguousarray(shared["sint"][:, q0:q0 + R]),
            **shared))

    res = run_bass_kernel_spmd(nc, in_maps, core_ids=list(range(N_CORES)))
    out = np.concatenate([res.results[c]["out"] for c in range(N_CORES)], axis=0)
    return out.reshape(b, n, d).astype(np.float32)
